# revision 1
# baseline (speedup 1.0000x reference)
"""Trainium2 Bass kernel for nn_FLossNoSoftMax (topk_masking).

Computes  -sum_b mean_v[(1-mask)*log(1-x)]  where mask marks the top-c
entries per row of x [2048, 50257] f32.

Math: per row  loss_b = (S_b - T_b)/V  with
  S_b = sum_v log(1-x[b,v])
  T_b = sum over the c largest values m of log(1-m)   (multiset, tie-exact)
result = -sum_b loss_b.

Device kernel (per core, 256 rows): stream [128 x F] chunks; scalar engine
computes Ln(1-x) with fused per-partition accumulation (-> S), vector engine
computes per-chunk top-8 values (InstMax); chunk top-8s are merged with one
final InstMax, giving the exact global top-8 multiset per row, whose first
c entries yield T.  Output: per-row (S_b - T_b); host does the final
-sum/V in float64.

Sharding: data-parallel over the batch dim, 256 rows per core on 8 cores.
"""

import sys

sys.path.insert(0, "/opt/trn_rl_repo")

import numpy as np

from concourse import bacc, bass, mybir, tile
from concourse.bass_utils import run_bass_kernel_spmd
from concourse.vector_clock import ScopedClock


def _ensure_axon_hooks():
    """The agent image lacks antenv.axon_hooks; run_bass_kernel_spmd imports
    it when tracing is requested (e.g. BASS_TRACE=1). Provide the module and
    wire the ctypes NTFF hook so tracing works instead of crashing."""
    try:
        import antenv.axon_hooks  # noqa: F401

        return
    except ImportError:
        pass
    import types

    try:
        import antenv
    except ImportError:
        return
    mod = types.ModuleType("antenv.axon_hooks")
    store = {"h": None}
    mod.set_axon_ntff_profile_hook = lambda h: store.__setitem__("h", h)
    mod.get_axon_ntff_profile_hook = lambda: store.get("h")
    sys.modules["antenv.axon_hooks"] = mod
    antenv.axon_hooks = mod
    try:
        from trn_agent_boot.trn_boot import _ntff_profile_via_ctypes

        mod.set_axon_ntff_profile_hook(
            _ntff_profile_via_ctypes("/opt/axon/libaxon_pjrt.so")
        )
        from concourse import bass_utils as _bu

        _bu.upload_artifacts = lambda d: "local://" + d
    except Exception:
        pass


_ensure_axon_hooks()


def _light_drain_and_barrier(self, tick_clock, wait_clock):
    # Tile's stock kernel tail runs two full all-engine barriers whose
    # GpSimd leg does an expensive dge_drain (~5-7us). All SWDGE loads are
    # provably retired here (their consumers gate the finals), so drain
    # every engine except GpSimd and use sem-only barriers instead.
    nc = self.nc
    drain_inst = nc.sync.drain()
    wait_clock.add_sem_waits(
        drain_inst.ins, ScopedClock({None: tick_clock.global_clock})
    )
    gp = nc.gpsimd.engine
    for eng_type, eng in nc.engines.items():
        if eng_type == gp:
            continue
        d = mybir.InstDrain(
            name=nc.get_next_instruction_name(), ins=[], outs=[],
            bass_is_fusable=False,
        )
        d.engine = eng_type
        eng.add_instruction(d)
    nc.all_engine_barrier(sem_only=True)
    popped = nc._tile_sem_poison_stack.pop()
    assert popped is self._sem_poison
    # Inline clear_and_free_semaphores, but run the DGE reset (gpsimd) and
    # the sem value clear (sync) on different engines so they overlap.
    sems = list(self.sems.allocated().values())
    if sems:
        sem_nums = [
            s.num if isinstance(s, bass.SemaphoreHandle) else s for s in sems
        ]
        for sem_range in bass.compact_to_ranges(sem_nums):
            assert nc._state.free_isdisjoint(sem_range)
            nc.gpsimd.dma_reset(sem_range)
            nc.sync.sem_clear(sem_range)
        nc._state.prepend_free_semaphores(sem_nums)
        for poison_set in nc._tile_sem_poison_stack:
            poison_set.update(sem_nums)
    nc.all_engine_barrier(sem_only=True)


tile.TileContext._drain_and_barrier = _light_drain_and_barrier

B, V = 2048, 50257
N_CORES = 8
ROWS_PER_CORE = B // N_CORES  # 256
P = 128
BLOCKS = ROWS_PER_CORE // P  # 2
F = 3072
NFULL = V // F  # 16
REM = V - NFULL * F  # 1105
NCHUNK = NFULL + 1  # 17

f32 = mybir.dt.float32
Ln = mybir.ActivationFunctionType.Ln
AX = mybir.AxisListType.X

_cache: dict = {}


def _build(top_c: int) -> bass.Bass:
    nc = bacc.Bacc("TRN2", target_bir_lowering=False)
    x = nc.dram_tensor("x", [ROWS_PER_CORE, V], f32, kind="ExternalInput")
    # out[p, blk] = S - T for row blk*128 + p
    out = nc.dram_tensor("out", [P, BLOCKS], f32, kind="ExternalOutput")

    with tile.TileContext(nc) as tc:
        with (
            tc.tile_pool(name="xp", bufs=8) as xp,
            tc.tile_pool(name="yp", bufs=3) as yp,
            tc.tile_pool(name="st", bufs=2) as st,
            tc.tile_pool(name="rp", bufs=1) as rp,
        ):
            res_all = rp.tile([P, BLOCKS], f32, tag="res_all")
            # DVE-initialized bias tile: keeps the activation-bias const off
            # the Pool-engine prologue, which delays the first load descgen.
            bias_t = rp.tile([P, 1], f32, tag="bias_t")
            nc.vector.memset(bias_t[:], 1.0)
            for blk in range(BLOCKS):
                rows = slice(blk * P, (blk + 1) * P)
                s_parts = st.tile([P, NCHUNK], f32, tag="s_parts")
                top8s = st.tile([P, 8 * NCHUNK], f32, tag="top8s")
                for c in range(NCHUNK):
                    sz = F if c < NFULL else REM
                    xt = xp.tile([P, sz], f32, tag="xt")
                    nc.gpsimd.dma_start(out=xt[:], in_=x[rows, c * F : c * F + sz])
                    yt = yp.tile([P, sz], f32, tag="yt")
                    nc.scalar.activation(
                        yt[:],
                        xt[:],
                        Ln,
                        bias=bias_t[:, 0:1],
                        scale=-1.0,
                        accum_out=s_parts[:, c : c + 1],
                    )
                    nc.vector.max(top8s[:, 8 * c : 8 * (c + 1)], xt[:])

                m8 = st.tile([P, 8], f32, tag="m8")
                nc.vector.max(m8[:], top8s[:])
                lnm = st.tile([P, top_c], f32, tag="lnm")
                t_sum = st.tile([P, 1], f32, tag="t_sum")
                nc.scalar.activation(
                    lnm[:], m8[:, :top_c], Ln, bias=bias_t[:, 0:1], scale=-1.0,
                    accum_out=t_sum[:],
                )
                s_tot = st.tile([P, 1], f32, tag="s_tot")
                nc.vector.reduce_sum(s_tot[:], s_parts[:], axis=AX)
                nc.vector.tensor_sub(
                    res_all[:, blk : blk + 1], s_tot[:], t_sum[:]
                )
            nc.sync.dma_start(out=out[:], in_=res_all[:])
    nc.compile()
    return nc


def _get(top_c: int) -> bass.Bass:
    if top_c not in _cache:
        _cache[top_c] = _build(top_c)
    return _cache[top_c]


def _run(output: np.ndarray, top_c: int, **spmd_kwargs):
    assert 1 <= top_c <= 8, f"kernel supports top_c in [1,8], got {top_c}"
    x = np.ascontiguousarray(np.asarray(output, dtype=np.float32))
    assert x.shape == (B, V), x.shape
    nc = _get(top_c)
    in_maps = [
        {"x": x[i * ROWS_PER_CORE : (i + 1) * ROWS_PER_CORE]} for i in range(N_CORES)
    ]
    res = run_bass_kernel_spmd(nc, in_maps, list(range(N_CORES)), **spmd_kwargs)
    parts = np.concatenate([r["out"].reshape(-1) for r in res.results])
    total = -np.sum(parts.astype(np.float64)) / V
    return np.float32(total), res


def kernel(top_c, output) -> np.ndarray:
    val, _ = _run(output, int(top_c))
    return np.array(val, dtype=np.float32)

